# revision 9
# baseline (speedup 1.0000x reference)
"""Trainium2 Bass kernel for DatasetIndexedTopK (streaming top-k retrieval).

Problem: scores = Q @ C^T with Q [512, 128], C [1M, 128]; return per-query
top-100 (scores, ids), matching jax.lax.top_k semantics (ties -> lower id).

Design (8-way shard over candidates, 131072 per core):
  The score volume per core is 4 query-chunks x 131072 candidates = 524288
  elements per SBUF partition; the bottleneck is draining it from PSUM
  (DVE tensor_reduce is 1x-rate, ~0.96 GHz).  v2 splits the drain between
  two engines working in parallel:

    span 0 of each 8192-wide tile (2048 cols):  DVE tensor_reduce max over
        innermost 32 directly from PSUM -> bf16 cell-max, cells = 32
        consecutive candidates.                         (~1.04 ns/el on DVE)
    spans 1..3: ACT (scalar engine) copies PSUM f32 -> SBUF bf16
        (~0.83 ns/el on ACT), then DVE folds with tensor_max (bf16 SBUF
        hits the 2x perf mode, ~0.52 ns/el) into a running per-column max
        across the 16 tiles -> cells = 16 candidates strided by 8192.

  Only cell maxima leave the device: out_cm [512, NCELL] bf16 per core
  (region A: 1024 32-member cells per chunk; region B: 6144 16-member
  cells per chunk).

  Host: concat the 8 cores' summaries; the kth-largest cell-max vk bounds
  the true s_k within the (matmul + bf16-rounding) margin, so selecting
  cells >= vk - MARGIN provably covers the true top-k.  Gather members of
  selected cells, rescore exactly in fp32, take exact top-k with the
  reference tie order (score desc, id asc).
"""

import numpy as np

P = 128                 # SBUF partitions / queries per chunk
D = 128                 # embedding dim (contraction)
Q = 512                 # queries
NCORES = 8
NCAND_TOTAL = 256 * 4096
NCAND = NCAND_TOTAL // NCORES    # 131072 candidates per core
CTILE = 8192            # candidate columns per DMA tile
NTILE = NCAND // CTILE  # 16
SPAN = 2048             # columns per PSUM span (4 banks)
NSPAN = CTILE // SPAN   # 4 spans per tile
SKIP_TILES = (5, 11)    # tiles whose span 0 goes to the ACT path (d-dither)
N_B = 4                 # fold slots (spans 1-3 always; slot 3 = span 0 of SKIP)
CELL_A = 32             # members per A-cell (consecutive)
CELL_B = NTILE          # members per B-cell (strided by CTILE)
NTILE_A = NTILE - len(SKIP_TILES)
NA = NTILE_A * (SPAN // CELL_A)        # A-cells per chunk  (896)
NB = N_B * SPAN                        # B-cells per chunk  (8192)
NCELL = NA + NB                        # summary width per chunk (9088)
NCHUNK = Q // P         # 4 query chunks
MARGIN = 1.0            # cell-selection slack (>> mm err + 2x bf16 ulp)

_CACHE = {}


def _build_bass(repeat=1):
    import concourse.bacc as bacc
    import concourse.mybir as mybir
    from concourse.tile import TileContext
    from contextlib import ExitStack

    f32 = mybir.dt.float32
    bf16 = mybir.dt.bfloat16
    ncell_span = SPAN // CELL_A        # 64

    nc = bacc.Bacc()
    qT = nc.declare_dram_parameter("qT", [D, Q], bf16, isOutput=False)
    candT = nc.declare_dram_parameter("candT", [D, NCAND], bf16, isOutput=False)
    out_cm = nc.declare_dram_parameter("out_cm", [Q, NCELL], bf16, isOutput=True)

    with ExitStack() as ctx:
        tc = ctx.enter_context(TileContext(nc))
        qpool = ctx.enter_context(tc.tile_pool(name="q", bufs=1))
        cpool = ctx.enter_context(tc.tile_pool(name="cand", bufs=3))
        pspool = ctx.enter_context(tc.tile_pool(name="ps", bufs=2, space="PSUM"))
        apool = ctx.enter_context(tc.tile_pool(name="accA", bufs=1))
        bpool = ctx.enter_context(tc.tile_pool(name="accB", bufs=1))
        stgpool = ctx.enter_context(tc.tile_pool(name="stg", bufs=2))

        qsb = qpool.tile([D, Q], bf16, tag="qsb")
        nc.sync.dma_start(qsb[:], qT[:])

        # A-region cell maxima: [128, NCHUNK * NA] bf16
        SA = apool.tile([P, NCHUNK * NA], bf16, tag="SA")
        # B-region running maxima: one [128, N_B, 64, 32] tile per chunk
        accB = [
            bpool.tile([P, N_B, ncell_span, CELL_A], bf16,
                       tag=f"accB_{qc}", name=f"accB_{qc}")
            for qc in range(NCHUNK)
        ]

        a_idx = {t: i for i, t in enumerate(
            t for t in range(NTILE) if t not in SKIP_TILES)}

        for t in range(repeat * NTILE):
            t = t % NTILE
            is_skip = t in SKIP_TILES
            first_skip = t == SKIP_TILES[0]
            nfold = 4 if (is_skip and not first_skip) else 3
            ct = cpool.tile([D, CTILE], bf16, tag="cand")
            nc.sync.dma_start(ct[:], candT[:, t * CTILE:(t + 1) * CTILE])
            for qc in range(NCHUNK):
                stg = None
                for sp in range(NSPAN):
                    ps = pspool.tile([P, ncell_span, CELL_A], f32, tag="ps")
                    for j in range(SPAN // 512):
                        col = sp * SPAN + j * 512
                        npc = 512 // CELL_A
                        nc.tensor.matmul(
                            ps[:, j * npc:(j + 1) * npc, :],
                            lhsT=qsb[:, qc * P:(qc + 1) * P],
                            rhs=ct[:, col: col + 512],
                            start=True,
                            stop=True,
                        )
                    if sp == 0 and not is_skip:
                        so = qc * NA + a_idx[t] * ncell_span
                        nc.vector.tensor_reduce(
                            out=SA[:, so:so + ncell_span], in_=ps[:],
                            axis=mybir.AxisListType.X, op=mybir.AluOpType.max,
                        )
                    else:
                        sl = 3 if sp == 0 else sp - 1
                        init = (t == 0) if sl < 3 else first_skip
                        if init:
                            nc.scalar.activation(
                                accB[qc][:, sl], ps[:],
                                mybir.ActivationFunctionType.Copy,
                            )
                        else:
                            if stg is None:
                                stg = stgpool.tile(
                                    [P, nfold, ncell_span, CELL_A], bf16,
                                    tag=f"stg{nfold}", name=f"stg{nfold}")
                            nc.scalar.activation(
                                stg[:, sl], ps[:],
                                mybir.ActivationFunctionType.Copy,
                            )
                if stg is not None:
                    # one 2x-rate fold for all staged spans at once
                    nc.vector.tensor_max(
                        accB[qc][:, :nfold], accB[qc][:, :nfold], stg[:])

        for qc in range(NCHUNK):
            nc.sync.dma_start(
                out_cm[qc * P:(qc + 1) * P, :NA],
                SA[:, qc * NA:(qc + 1) * NA],
            )
            nc.sync.dma_start(
                out_cm[qc * P:(qc + 1) * P, NA:],
                accB[qc][:],
            )
    nc.compile()
    return nc


def _get_bass():
    if "nc" not in _CACHE:
        _CACHE["nc"] = _build_bass()
    return _CACHE["nc"]


def _cell_member_tables():
    """POS_A [NA, 32] and POS_B_pad [NB, 32] of core-local candidate columns
    (POS_B rows are padded to 32 with a huge sentinel)."""
    nonskip = np.array([t for t in range(NTILE) if t not in SKIP_TILES])
    i = np.arange(NA)
    t = nonskip[i // (SPAN // CELL_A)]
    c32 = i % (SPAN // CELL_A)
    baseA = t * CTILE + c32 * CELL_A          # span 0 of non-skip tiles
    POS_A = baseA[:, None] + np.arange(CELL_A)[None, :]

    BIG = 1 << 40
    POS_B_pad = np.full((NB, CELL_A), BIG, dtype=np.int64)
    j = np.arange(NB)
    sl = j // SPAN
    off = j % SPAN
    m3 = sl < 3                                # slots 0-2: spans 1-3, 16 tiles
    POS_B_pad[np.ix_(m3.nonzero()[0], np.arange(NTILE))] = (
        np.arange(NTILE)[None, :] * CTILE
        + ((sl[m3] + 1) * SPAN + off[m3])[:, None])
    m4 = ~m3                                   # slot 3: span 0 of skip tiles
    POS_B_pad[np.ix_(m4.nonzero()[0], np.arange(len(SKIP_TILES)))] = (
        np.array(SKIP_TILES)[None, :] * CTILE + off[m4][:, None])
    return POS_A.astype(np.int64), POS_B_pad


def kernel(query_embeddings, candidate_embeddings, candidate_indices, k):
    from concourse.bass_utils import run_bass_kernel_spmd

    q = np.ascontiguousarray(np.asarray(query_embeddings, dtype=np.float32))
    c = np.asarray(candidate_embeddings, dtype=np.float32).reshape(NCAND_TOTAL, D)
    ids_flat = np.asarray(candidate_indices).reshape(-1)
    k = int(k)
    assert k <= 1024

    import ml_dtypes
    bf16 = ml_dtypes.bfloat16
    qT = np.ascontiguousarray(q.T).astype(bf16)          # [128, 512]
    cT = np.ascontiguousarray(c.T.astype(bf16))          # [128, 1048576]
    in_maps = []
    for core in range(NCORES):
        in_maps.append({
            "qT": qT,
            "candT": cT[:, core * NCAND:(core + 1) * NCAND],
        })

    nc = _get_bass()
    res = run_bass_kernel_spmd(nc, in_maps, core_ids=list(range(NCORES))).results

    # ---- host: exact top-k from cell-max summaries ----
    cm = np.concatenate(
        [res[core]["out_cm"].astype(np.float32) for core in range(NCORES)],
        axis=1,
    )                                                    # [512, 8*NCELL]
    vk = np.partition(cm, -k, axis=1)[:, -k]             # kth-largest cell max
    tau = vk - MARGIN
    counts = (cm >= tau[:, None]).sum(axis=1)
    K = int(counts.max())
    sel_cells = np.argpartition(-cm, K - 1, axis=1)[:, :K]   # [512, K]

    POS_A, POS_B_pad = _cell_member_tables()
    SENT = NCAND_TOTAL                                   # dummy candidate id
    core_of = sel_cells // NCELL
    local = sel_cells - core_of * NCELL
    mA = local < NA
    pos = np.empty((Q, K, CELL_A), dtype=np.int64)
    pos[mA] = core_of[mA][:, None] * NCAND + POS_A[local[mA]]
    mB = ~mA
    pos[mB] = core_of[mB][:, None] * NCAND + POS_B_pad[local[mB] - NA]
    np.minimum(pos, SENT, out=pos)
    pos = pos.reshape(Q, K * CELL_A)

    c_ext = np.vstack([c, np.zeros((1, D), dtype=np.float32)])
    out_scores = np.empty((Q, k), dtype=np.float32)
    out_pos = np.empty((Q, k), dtype=np.int64)
    QB = 64                                              # query batch (memory cap)
    for q0 in range(0, Q, QB):
        q1 = min(q0 + QB, Q)
        sel = c_ext[pos[q0:q1]]                          # [qb, K*32, 128]
        sc = np.einsum("qnd,qd->qn", sel, q[q0:q1], optimize=True)
        for qi in range(q0, q1):
            row = sc[qi - q0]
            p = pos[qi]
            # exact order among a slightly larger head to honor tie-break
            head = np.argpartition(-row, min(k + 32, row.size - 1))[:k + 32]
            order = head[np.lexsort((p[head], -row[head]))][:k]
            out_scores[qi] = row[order]
            out_pos[qi] = p[order]

    ids_ext = np.concatenate([ids_flat, np.zeros(1, dtype=ids_flat.dtype)])
    out_ids = ids_ext[out_pos].astype(ids_flat.dtype)
    return out_scores, out_ids
